# revision 4
# baseline (speedup 1.0000x reference)
"""Trainium2 Bass kernel v2 for the 2-layer DPHGNN + hyperconv GNN stack.

Differences from the v1 baseline:
- node-side intermediates (x_init, transposed h) live in SBUF, bf16
- e2v node epilogue fused into the scatter pass (no npart DRAM round trip)
- dead h stores removed
- ReduceScatter/AllGather split into 4 pipelined chunks over a padded
  160-tile edge space (edge ownership remapped chunk-major)
- dense weights/lhsT in bf16, one-hot A matrices built in bf16 (2x DVE)
- table writes batched 4 node tiles per DMA; rse/ytab marked Shared
- deeper gather buffering, loads issued from the scalar (ACT) HWDGE queue
"""

import sys
from contextlib import ExitStack

for _p in ("/opt/trn_rl_repo",):
    if _p not in sys.path:
        sys.path.append(_p)

import numpy as np

import concourse.bass as bass
import concourse.bacc as bacc
import concourse.mybir as mybir
import concourse.tile as tile
from concourse.bass_utils import run_bass_kernel_spmd
from concourse.masks import make_identity

F32 = mybir.dt.float32
BF16 = mybir.dt.bfloat16
I16 = mybir.dt.int16
AF = mybir.ActivationFunctionType

NEG_SLOPE = 0.2
P = 128
NCORES = 8
GQ = 4          # SWDGE queues
NI = 1024       # rows per dma_gather call (hard ucode limit)
WCH = 8         # chunks per gather call / A-build batch
PSW = 2         # PSUM tiles per scatter mega-window
GB_V = 8        # gather bufs, v2e stream
GB_E = 6        # gather bufs, e2v stream
RSK = 4         # ReduceScatter/AllGather pipeline chunks
RS_LAG = 10     # tiles of lag before firing an RS chunk

N_N, N_M = 50000, 20000
NS = N_N // NCORES               # 6250 nodes per core
NT_V = (NS + P - 1) // P         # 49
NT_E = 160                       # padded edge tiles (20480 rows)
ME = NT_E * P                    # 20480
CHT = NT_E // RSK                # 40 edge tiles per RS chunk
CHROWS = CHT * P                 # 5120 global rows per chunk
OWNR = CHROWS // NCORES          # 640 rows owned per core per chunk
MS_OWN = OWNR * RSK              # 2560 owned rows per core
NT_MS = MS_OWN // P              # 20 owned tiles
OWNT = OWNR // P                 # 5 owned tiles per chunk


def _wrap_idx(flat):
    L = len(flat)
    assert L % 16 == 0
    blk = np.asarray(flat, np.int16).reshape(-1, 16).T.copy()
    return np.ascontiguousarray(np.tile(blk, (8, 1)))


def _build_stream(dst, src_idx, n_tiles, cpt):
    """Destination-sorted, per-tile 128-padded entry stream."""
    order = np.argsort(dst, kind="stable")
    dsts = np.asarray(dst)[order]
    srcs = np.asarray(src_idx)[order]
    tile_of = dsts // P
    counts = np.bincount(tile_of, minlength=n_tiles)
    base = np.concatenate([[0], np.cumsum(cpt * P)])
    L = int(base[-1])
    gidx = np.zeros(L, np.int64)
    ec = -np.ones(L, np.float32)
    starts = np.concatenate([[0], np.cumsum(counts)])
    off = np.arange(len(dsts)) - starts[tile_of]
    slot = base[tile_of] + off
    gidx[slot] = srcs
    ec[slot] = dsts - tile_of * P
    return gidx, ec


def _own_rows(c):
    """Global edge rows owned by core c (chunk-major RS layout)."""
    rows = []
    for k in range(RSK):
        r0 = k * CHROWS + c * OWNR
        rows.append(np.arange(r0, r0 + OWNR))
    return np.concatenate(rows)


def _prep(inputs):
    V = np.asarray(inputs["V"]).astype(np.int64)
    E = np.asarray(inputs["E"]).astype(np.int64)
    X = np.asarray(inputs["X"], np.float32)
    S = np.asarray(inputs["S"], np.float32)

    deg_v = np.bincount(V, minlength=N_N).astype(np.float64)
    cnt_e = np.bincount(E, minlength=N_M).astype(np.float64)
    deginv = np.where(deg_v > 0, 1.0 / np.maximum(deg_v, 1.0), 0.0)
    De = np.zeros(N_M, np.float64)
    np.add.at(De, E, deg_v[V])
    De = De / (cnt_e + 1.0)
    De_inv = np.where(De > 0, De ** -0.5, 1.0)
    coef_e = np.where(cnt_e > 0, De_inv / np.maximum(cnt_e, 1.0), 0.0)
    Dv_inv = np.where(deg_v > 0, deg_v ** -0.5, 0.0)

    owner = V // NS
    v2e_raw, e2v_raw = [], []
    for c in range(NCORES):
        m = owner == c
        Vl = V[m] - c * NS
        Ee = E[m]
        v2e_raw.append((Ee, Vl))
        e2v_raw.append((Vl, Ee))

    def caps(raw, n_tiles, min1=True):
        cpt = None
        for dst, _ in raw:
            counts = np.bincount(np.asarray(dst) // P, minlength=n_tiles)
            c1 = (counts + P - 1) // P
            if min1:
                c1 = np.maximum(1, c1)
            cpt = c1 if cpt is None else np.maximum(cpt, c1)
        return cpt

    # e2v as one stream (A = whole ytab) plus an unused empty B; the split
    # variant measured slower, so B is disabled.
    e2v_a = e2v_raw
    e2v_b = [(np.zeros(0, np.int64), np.zeros(0, np.int64))
             for _ in range(NCORES)]

    cpt_v2e = caps(v2e_raw, NT_E)
    cpt_e2v_a = caps(e2v_a, NT_V, min1=True)
    cpt_e2v_b = caps(e2v_b, NT_V, min1=False)
    Lv = int(np.sum(cpt_v2e) * P)
    LeA = int(np.sum(cpt_e2v_a) * P)
    LeB = int(np.sum(cpt_e2v_b) * P)
    LvP = ((Lv + NI - 1) // NI) * NI
    LeAP = ((LeA + NI - 1) // NI) * NI
    LeBP = max(NI, ((LeB + NI - 1) // NI) * NI)

    def pad_stream(g, ec, LP):
        gi = np.full(LP, -1, np.int64)
        gi[: len(g)] = g
        ecp = np.full(LP, -1.0, np.float32)
        ecp[: len(ec)] = ec
        ecb = ecp.astype(np.dtype("bfloat16"))
        return gi, np.ascontiguousarray(ecb.reshape(-1, P).T)

    cores = []
    for c in range(NCORES):
        gv, ecv = _build_stream(*v2e_raw[c], NT_E, cpt_v2e)
        gea, ecea = _build_stream(e2v_a[c][0], e2v_a[c][1], NT_V, cpt_e2v_a)
        geb, eceb = _build_stream(e2v_b[c][0], e2v_b[c][1], NT_V, cpt_e2v_b)
        gv_p, ecv_2d = pad_stream(gv, ecv, LvP)
        gea_p, ecea_2d = pad_stream(gea, ecea, LeAP)
        geb_p, eceb_2d = pad_stream(geb, eceb, LeBP)
        cores.append(dict(gv_p=gv_p, ecv_2d=ecv_2d,
                          gea_p=gea_p, ecea_2d=ecea_2d,
                          geb_p=geb_p, eceb_2d=eceb_2d))

    def regs(L, LP):
        return [int(max(0, min(L - k * NI, NI))) for k in range(LP // NI)]

    regs_v = regs(Lv, LvP)
    regs_ea = regs(LeA, LeAP)
    regs_eb = regs(LeB, LeBP)

    bf = np.dtype("bfloat16")
    g = lambda k: np.asarray(inputs[k], np.float32)
    W = {}
    for l in range(2):
        Wv, bv, a = g(f"Wv{l}"), g(f"bv{l}"), g(f"a{l}")
        Wx, bx = g(f"Wx{l}"), g(f"bx{l}")
        Wt, bt = g(f"Wt{l}"), g(f"bt{l}")
        Wva = np.concatenate([Wv, (Wv @ a)[:, None]], axis=1)
        bva = np.concatenate([bv, [float(bv @ a)]])
        Wt_top, Wt_bot = Wt[:256], Wt[256:]
        btf = bt - Wt_top.sum(axis=0)
        nh = Wva.shape[0] // P
        for hi in range(nh):
            W[f"Wva{l}h{hi}"] = np.ascontiguousarray(
                Wva[hi * P:(hi + 1) * P]).astype(bf)
            W[f"Wx{l}h{hi}"] = np.ascontiguousarray(
                Wx[hi * P:(hi + 1) * P]).astype(bf)
        W[f"bva{l}"] = np.tile(bva[None, :].astype(np.float32), (P, 1))
        W[f"bx{l}"] = np.tile((bx - 1.0)[None, :], (P, 1))
        W[f"Wt{l}h0"] = np.ascontiguousarray(Wt_top[:128]).astype(bf)
        W[f"Wt{l}h1"] = np.ascontiguousarray(Wt_top[128:]).astype(bf)
        W[f"Wt{l}bot"] = np.ascontiguousarray(Wt_bot).astype(bf)
        W[f"bt{l}"] = np.tile(btf[None, :].astype(np.float32), (P, 1))
    Wf = g("Wf")
    W["Wfh0"] = np.ascontiguousarray(Wf[:128]).astype(bf)
    W["Wfh1"] = np.ascontiguousarray(Wf[128:]).astype(bf)
    W["bf"] = np.tile(g("bf")[None, :], (P, 1))

    iota = np.tile(np.arange(P, dtype=np.float32)[None, :], (P, 1))
    iota_rep = np.ascontiguousarray(
        np.broadcast_to(iota[:, None, :], (P, WCH, P))).astype(bf)

    def cols(arr, n_tiles):
        out = np.zeros((P, n_tiles), np.float32)
        a = np.asarray(arr, np.float32)
        for t in range(n_tiles):
            seg = a[t * P:(t + 1) * P]
            out[: len(seg), t] = seg
        return out

    in_maps = []
    for c in range(NCORES):
        d = cores[c]
        own = _own_rows(c)
        own_real = own[own < N_M]
        ST_own = np.zeros((MS_OWN, 64), np.float32)
        ST_own[own < N_M] = S[own_real]
        coef_own = np.zeros(MS_OWN, np.float32)
        coef_own[own < N_M] = coef_e[own_real]
        im = dict(
            XT=np.ascontiguousarray(X[c * NS:(c + 1) * NS].T).astype(bf),
            ST=np.ascontiguousarray(ST_own.T).astype(bf),
            gv_idx=_wrap_idx(d["gv_p"]),
            gea_idx=_wrap_idx(d["gea_p"]), geb_idx=_wrap_idx(d["geb_p"]),
            ec_v=d["ecv_2d"], ec_ea=d["ecea_2d"], ec_eb=d["eceb_2d"],
            iota_rep=iota_rep,
            deginv_c=cols(deginv[c * NS:(c + 1) * NS], NT_V),
            dvinv_c=cols(Dv_inv[c * NS:(c + 1) * NS], NT_V),
            coef_c=cols(coef_own, NT_MS),
        )
        im.update(W)
        in_maps.append(im)

    meta = dict(cpt_v2e=[int(x) for x in cpt_v2e],
                cpt_e2v_a=[int(x) for x in cpt_e2v_a],
                cpt_e2v_b=[int(x) for x in cpt_e2v_b],
                LvP=LvP, LeAP=LeAP, LeBP=LeBP,
                regs_v=regs_v, regs_ea=regs_ea, regs_eb=regs_eb)
    return in_maps, meta


# ---------------------------------------------------------------------------

def build_program(meta):
    ESV, ESE, ESH = 384, 256, 128

    nc = bacc.Bacc("TRN2", target_bir_lowering=False, debug=False,
                   num_devices=NCORES, num_swdge_queues=GQ)

    def din(name, shape, dt=F32):
        return nc.dram_tensor(name, shape, dt, kind="ExternalInput")

    XT = din("XT", [P, NS], BF16)
    ST = din("ST", [64, MS_OWN], BF16)
    gv_idx = din("gv_idx", [P, meta["LvP"] // 16], I16)
    gea_idx = din("gea_idx", [P, meta["LeAP"] // 16], I16)
    geb_idx = din("geb_idx", [P, meta["LeBP"] // 16], I16)
    nch_v = meta["LvP"] // P
    nch_ea = meta["LeAP"] // P
    nch_eb = meta["LeBP"] // P
    ec_v = din("ec_v", [P, nch_v], BF16)
    ec_ea = din("ec_ea", [P, nch_ea], BF16)
    ec_eb = din("ec_eb", [P, nch_eb], BF16)
    iota_rep = din("iota_rep", [P, WCH, P], BF16)
    deginv_c = din("deginv_c", [P, NT_V])
    dvinv_c = din("dvinv_c", [P, NT_V])
    coef_c = din("coef_c", [P, NT_MS])
    wnames = (["Wva0h0", "Wx0h0", "Wva1h0", "Wva1h1", "Wx1h0", "Wx1h1",
               "Wt0h0", "Wt0h1", "Wt1h0", "Wt1h1", "Wfh0", "Wfh1",
               "Wt0bot", "Wt1bot"],
              ["bva0", "bx0", "bva1", "bx1", "bt0", "bt1", "bf"])
    wshapes = dict(Wva0h0=[P, 257], Wx0h0=[P, 256],
                   Wva1h0=[P, 257], Wva1h1=[P, 257],
                   Wx1h0=[P, 256], Wx1h1=[P, 256],
                   Wt0h0=[P, 256], Wt0h1=[P, 256],
                   Wt1h0=[P, 256], Wt1h1=[P, 256],
                   Wfh0=[P, 128], Wfh1=[P, 128],
                   Wt0bot=[64, 256], Wt1bot=[64, 256],
                   bva0=[P, 257], bx0=[P, 256], bva1=[P, 257], bx1=[P, 256],
                   bt0=[P, 256], bt1=[P, 256], bf=[P, 128])
    Wd = {k: din(k, wshapes[k], BF16) for k in wnames[0]}
    Wd.update({k: din(k, wshapes[k], F32) for k in wnames[1]})

    yout = nc.dram_tensor("yout", [NS, 128], F32, kind="ExternalOutput")

    rg = [list(range(NCORES))]

    with tile.TileContext(nc) as tc:
        ctx = ExitStack()
        sbuf = ctx.enter_context(tc.tile_pool(name="sbuf", bufs=2))
        psum = ctx.enter_context(tc.tile_pool(name="psum", bufs=2, space="PSUM"))
        dram = ctx.enter_context(tc.tile_pool(name="dram", bufs=1, space="DRAM"))
        cons = ctx.enter_context(tc.tile_pool(name="cons", bufs=1))

        iota_t = cons.tile([P, WCH, P], BF16, name="iota_t")
        nc.scalar.dma_start(iota_t[:], iota_rep[:])
        ident = cons.tile([P, P], F32, name="ident")
        make_identity(nc, ident[:])
        wt = {}
        for k, h in Wd.items():
            t = cons.tile(list(h.shape), h.dtype, name=f"w_{k}")
            nc.scalar.dma_start(t[:], h[:])
            wt[k] = t
        st_t = cons.tile([64, MS_OWN], BF16, name="st_t")
        nc.sync.dma_start(st_t[:], ST[:])
        ecv_t = cons.tile([P, nch_v], BF16, name="ecv_t")
        nc.scalar.dma_start(ecv_t[:], ec_v[:])
        ecea_t = cons.tile([P, nch_ea], BF16, name="ecea_t")
        nc.scalar.dma_start(ecea_t[:], ec_ea[:])
        eceb_t = cons.tile([P, nch_eb], BF16, name="eceb_t")
        nc.scalar.dma_start(eceb_t[:], ec_eb[:])
        gvi_t = cons.tile([P, meta["LvP"] // 16], I16, name="gvi_t")
        nc.sync.dma_start(gvi_t[:], gv_idx[:])
        geai_t = cons.tile([P, meta["LeAP"] // 16], I16, name="geai_t")
        nc.sync.dma_start(geai_t[:], gea_idx[:])
        gebi_t = cons.tile([P, meta["LeBP"] // 16], I16, name="gebi_t")
        nc.sync.dma_start(gebi_t[:], geb_idx[:])
        dgi_t = cons.tile([P, NT_V], F32, name="dgi_t")
        nc.scalar.dma_start(dgi_t[:], deginv_c[:])
        dvi_t = cons.tile([P, NT_V], F32, name="dvi_t")
        nc.scalar.dma_start(dvi_t[:], dvinv_c[:])
        cf_t = cons.tile([P, NT_MS], F32, name="cf_t")
        nc.scalar.dma_start(cf_t[:], coef_c[:])
        xt_t = cons.tile([P, NS], BF16, name="xt_t")
        nc.sync.dma_start(xt_t[:], XT[:])

        # SBUF-resident node-side state (bf16)
        xinit_sb = cons.tile([P, NT_V, 256], BF16, name="xinit_sb")
        hT = [cons.tile([P, NT_V * P], BF16, name=f"hT{hi}") for hi in range(2)]

        qctr = [0]

        def scatter_pass(streams, used_cols, n_tiles, on_tile):
            """Gather + one-hot-matmul segment sum over dest tiles.

            streams: list of dicts (in_ap, es, idx_t, ec_t, cpt, regs, tag,
            gb); chunks are consumed tile-major, streams in order within a
            tile.  on_tile(t, psum_ap, q, last_in_window, n_in_window) fires
            when tile t's accumulation is complete."""
            S = len(streams)
            chunk_lists = []
            for st in streams:
                tof = []
                for t, n in enumerate(st["cpt"]):
                    tof += [t] * n
                chunk_lists.append(tof)
            order = []
            ks = [0] * S
            for t in range(n_tiles):
                for s in range(S):
                    for _ in range(streams[s]["cpt"][t]):
                        order.append((s, ks[s]))
                        ks[s] += 1
            first_c, last_c = {}, {}
            for pos, (s, k) in enumerate(order):
                t = chunk_lists[s][k]
                first_c.setdefault(t, pos)
                last_c[t] = pos
            g_tiles = [[None] * len(st["regs"]) for st in streams]
            emitted = [0] * S

            def ensure_emitted(s, upto):
                st = streams[s]
                while emitted[s] <= min(upto, len(st["regs"]) - 1):
                    call = emitted[s]
                    if st["regs"][call] > 0:
                        gt = sbuf.tile([P, WCH, st["es"]], BF16,
                                       tag=st["ring"], bufs=st["gb"],
                                       name=f"g{st['tag']}_{call}")
                        nc.gpsimd.dma_gather(
                            out_ap=gt[:], in_ap=st["in_ap"],
                            idxs_ap=st["idx_t"][:, call * (NI // 16):
                                                (call + 1) * (NI // 16)],
                            num_idxs=NI, num_idxs_reg=st["regs"][call],
                            elem_size=st["es"], queue_num=qctr[0] % GQ)
                        qctr[0] += 1
                        g_tiles[s][call] = gt
                    emitted[s] += 1

            a_cur = [[None, -1] for _ in range(S)]
            mega = [None, -1]
            for pos, (s, k) in enumerate(order):
                st = streams[s]
                t = chunk_lists[s][k]
                call, j = k // WCH, k % WCH
                ensure_emitted(s, call + st["gb"] - 1)
                gt = g_tiles[s][call]
                if gt is None:
                    continue
                w = k // WCH
                if a_cur[s][1] != w:
                    ab = sbuf.tile([P, WCH, P], BF16, tag=f"A{s}", bufs=2,
                                   name=f"A{st['tag']}_{w}")
                    nc.vector.tensor_tensor(
                        out=ab[:],
                        in0=st["ec_t"][:, w * WCH:(w + 1) * WCH].to_broadcast(
                            [P, WCH, P]),
                        in1=iota_t[:],
                        op=mybir.AluOpType.is_equal)
                    a_cur[s] = [ab, w]
                mw = t // PSW
                if mega[1] != mw:
                    mega = [psum.tile([P, PSW, 512], F32, tag="ps", bufs=2,
                                      name=f"ps{st['tag']}_{mw}"), mw]
                pt = mega[0]
                q = t % PSW
                nc.tensor.matmul(
                    out=pt[:, q, 0:used_cols],
                    lhsT=a_cur[s][0][:, j, :],
                    rhs=gt[:, j, 0:used_cols],
                    start=(pos == first_c[t]), stop=(pos == last_c[t]))
                if pos == last_c[t]:
                    last_in_w = (t % PSW == PSW - 1) or (t == n_tiles - 1)
                    on_tile(t, pt, q, last_in_w, q + 1)

        def cc_emit(kind, op, ins, outs):
            # emit collectives from the (mostly idle) scalar engine so their
            # waits never stall the gather stream on GpSimd
            bass.BassGpSimd.collective_compute(
                nc.gpsimd, kind, op, replica_groups=rg, ins=ins, outs=outs)

        def elu_u(z_ap, w, cols, tag, i):
            """relu(z) + exp(min(z,0)) = elu(z) + 1."""
            mn = sbuf.tile([P, cols], F32, tag="mn", bufs=2, name=f"mn{tag}{i}")
            nc.vector.tensor_scalar_min(out=mn[:w], in0=z_ap, scalar1=0.0)
            ex = sbuf.tile([P, cols], F32, tag="ex", bufs=2, name=f"ex{tag}{i}")
            nc.scalar.activation(ex[:w], mn[:w], AF.Exp)
            rl = sbuf.tile([P, cols], F32, tag="rl", bufs=2, name=f"rl{tag}{i}")
            nc.vector.tensor_scalar_max(out=rl[:w], in0=z_ap, scalar1=0.0)
            u = sbuf.tile([P, cols], F32, tag="u", bufs=2, name=f"u{tag}{i}")
            nc.vector.tensor_add(u[:w], rl[:w], ex[:w])
            return u

        def dense_and_table(l, lhsT_of, table, with_score):
            """Per node tile: table row block + x_init slice (SBUF)."""
            nh = 1 if l == 0 else 2
            stg4 = [None]
            tcols = 257 if with_score else 128
            for t in range(NT_V):
                w = min(P, NS - t * P)
                halves = lhsT_of(t, w)
                pf = psum.tile([P, 512], F32, tag="pd", bufs=2, name=f"pf{l}_{t}")
                if with_score:
                    for hi in range(nh):
                        nc.tensor.matmul(out=pf[:w, 0:257], lhsT=halves[hi],
                                         rhs=wt[f"Wva{l}h{hi}"][:],
                                         start=(hi == 0), stop=(hi == nh - 1))
                else:
                    for hi in range(nh):
                        nc.tensor.matmul(out=pf[:w, 0:128], lhsT=halves[hi],
                                         rhs=wt[f"Wfh{hi}"][:],
                                         start=(hi == 0), stop=(hi == nh - 1))
                if stg4[0] is None:
                    stg4[0] = sbuf.tile([P, 4, tcols], BF16, tag="stg4", bufs=2,
                                        name=f"stg4{l}_{t}")
                j4 = t % 4
                if with_score:
                    F = sbuf.tile([P, 257], F32, tag="F", bufs=2,
                                  name=f"F{l}_{t}")
                    nc.vector.tensor_add(F[:w], pf[:w, 0:257], wt[f"bva{l}"][:w])
                    lr = sbuf.tile([P, 1], F32, tag="lr", bufs=2,
                                   name=f"lr{l}_{t}")
                    nc.vector.tensor_scalar_mul(out=lr[:w],
                                                in0=F[:w, 256:257],
                                                scalar1=NEG_SLOPE)
                    ew = sbuf.tile([P, 1], F32, tag="ew", bufs=2,
                                   name=f"ew{l}_{t}")
                    nc.vector.tensor_tensor(out=ew[:w], in0=F[:w, 256:257],
                                            in1=lr[:w],
                                            op=mybir.AluOpType.max)
                    nc.scalar.activation(ew[:w], ew[:w], AF.Exp)
                    nc.scalar.activation(stg4[0][:w, j4, 0:256],
                                         F[:w, 0:256], AF.Copy,
                                         scale=ew[:w, :])
                    nc.vector.tensor_copy(out=stg4[0][:w, j4, 256:257],
                                          in_=ew[:w, :])
                else:
                    nc.vector.tensor_add(stg4[0][:w, j4, 0:128],
                                         pf[:w, 0:128], wt["bf"][:w])
                if t % 4 == 3 or t == NT_V - 1:
                    nj = j4 + 1
                    r0 = (t - nj + 1) * P
                    nc.sync.dma_start(
                        out=table[r0:r0 + nj * P, 0:tcols].rearrange(
                            "(j p) c -> p j c", p=P),
                        in_=stg4[0][:, 0:nj, :])
                    stg4[0] = None
                if with_score:
                    pi = psum.tile([P, 512], F32, tag="pd", bufs=2,
                                   name=f"pi{l}_{t}")
                    for hi in range(nh):
                        nc.tensor.matmul(out=pi[:w, 0:256], lhsT=halves[hi],
                                         rhs=wt[f"Wx{l}h{hi}"][:],
                                         start=(hi == 0), stop=(hi == nh - 1))
                    nc.vector.tensor_add(xinit_sb[:w, t, :], pi[:w, 0:256],
                                         wt[f"bx{l}"][:w])

        def edge_epilogue_tile(l, g, rse, yin):
            """Process owned tile g (rows g*128..+128 of the rse shard)."""
            if True:
                r0 = g * P
                rt = sbuf.tile([P, 257], BF16, tag="rt", bufs=2,
                               name=f"rt{l}_{g}")
                nc.scalar.dma_start(rt[:], rse[r0:r0 + P, :])
                dc = sbuf.tile([P, 1], F32, tag="dc", bufs=2,
                               name=f"dc{l}_{g}")
                nc.vector.tensor_scalar_max(out=dc[:], in0=rt[:, 256:257],
                                            scalar1=1e-35)
                di = sbuf.tile([P, 1], F32, tag="di", bufs=2,
                               name=f"di{l}_{g}")
                nc.vector.reciprocal(di[:], dc[:])
                z = sbuf.tile([P, 256], F32, tag="z", bufs=2,
                              name=f"z{l}_{g}")
                nc.scalar.activation(z[:], rt[:, 0:256], AF.Copy,
                                     scale=di[:, :])
                u = elu_u(z[:], P, 256, f"ee{l}", g)
                uT = []
                for hi in range(2):
                    pT = psum.tile([P, P], F32, tag="pT", bufs=2,
                                   name=f"pT{l}_{g}_{hi}")
                    nc.tensor.transpose(out=pT[:, 0:P],
                                        in_=u[:, hi * P:(hi + 1) * P],
                                        identity=ident[:, :])
                    sT = sbuf.tile([P, P], BF16, tag="sT", bufs=2,
                                   name=f"sT{l}_{g}_{hi}")
                    nc.vector.tensor_copy(out=sT[:], in_=pT[:])
                    uT.append(sT)
                py = psum.tile([P, 512], F32, tag="pd", bufs=2,
                               name=f"py{l}_{g}")
                nc.tensor.matmul(out=py[:, 0:256],
                                 lhsT=st_t[:, g * P:(g + 1) * P],
                                 rhs=wt[f"Wt{l}bot"][:], start=True, stop=False)
                nc.tensor.matmul(out=py[:, 0:256], lhsT=uT[0][:],
                                 rhs=wt[f"Wt{l}h0"][:], start=False, stop=False)
                nc.tensor.matmul(out=py[:, 0:256], lhsT=uT[1][:],
                                 rhs=wt[f"Wt{l}h1"][:], start=False, stop=True)
                yt = sbuf.tile([P, 256], BF16, tag="yt", bufs=2,
                               name=f"yt{l}_{g}")
                nc.vector.tensor_add(yt[:], py[:, 0:256], wt[f"bt{l}"][:])
                nc.sync.dma_start(out=yin[r0:r0 + P, :], in_=yt[:])

        def dphgnn(l, lhsT_of):
            table = dram.tile([NT_V * P, ESV], BF16, name=f"T{l}")
            dense_and_table(l, lhsT_of, table, True)

            part = dram.tile([ME, 257], BF16, name=f"part{l}")
            rse = dram.tile([MS_OWN, 257], BF16, name=f"rse{l}")
            yin = dram.tile([MS_OWN, 256], BF16, name=f"yin{l}")
            ytab = dram.tile([ME, ESE], BF16, name=f"ytab{l}")
            pstg = [None]
            rs_done = [0]

            def v2e_tile(t, pt, q, last_in_w, n_in_w):
                if pstg[0] is None:
                    pstg[0] = sbuf.tile([P, PSW, 257], BF16, tag="pstg",
                                        bufs=4, name=f"pstg{l}_{t}")
                nc.vector.tensor_copy(out=pstg[0][:, q, :],
                                      in_=pt[:, q, 0:257])
                if last_in_w:
                    rows0 = (t - n_in_w + 1) * P
                    nc.sync.dma_start(
                        out=part[rows0:rows0 + n_in_w * P, :].rearrange(
                            "(j p) c -> p j c", p=P),
                        in_=pstg[0][:, 0:n_in_w, :])
                    pstg[0] = None
                if last_in_w:
                    while (rs_done[0] < RSK
                           and t >= (rs_done[0] + 1) * CHT - 1 + RS_LAG):
                        kc = rs_done[0]
                        cc_emit("ReduceScatter", mybir.AluOpType.add,
                                [part[kc * CHROWS:(kc + 1) * CHROWS, :]],
                                [rse[kc * OWNR:(kc + 1) * OWNR, :]])
                        rs_done[0] += 1

            scatter_pass([dict(in_ap=table[:], es=ESV, idx_t=gvi_t,
                               ec_t=ecv_t, cpt=meta["cpt_v2e"],
                               regs=meta["regs_v"], tag=f"v{l}", gb=GB_V, ring="gv")],
                         257, NT_E, v2e_tile)
            while rs_done[0] < RSK:
                kc = rs_done[0]
                cc_emit("ReduceScatter", mybir.AluOpType.add,
                        [part[kc * CHROWS:(kc + 1) * CHROWS, :]],
                        [rse[kc * OWNR:(kc + 1) * OWNR, :]])
                rs_done[0] += 1

            ag_done = [0]
            for g in range(NT_MS):
                edge_epilogue_tile(l, g, rse, yin)
                while (ag_done[0] < RSK
                       and (g + 1) * P >= (ag_done[0] + 1) * OWNR):
                    k = ag_done[0]
                    cc_emit("AllGather", mybir.AluOpType.bypass,
                            [yin[k * OWNR:(k + 1) * OWNR, :]],
                            [ytab[k * CHROWS:(k + 1) * CHROWS, :]])
                    ag_done[0] += 1

            def e2v_tile(t, pt, q, last_in_w, n_in_w):
                w = min(P, NS - t * P)
                z = sbuf.tile([P, 256], F32, tag="nz", bufs=2,
                              name=f"nz{l}_{t}")
                nc.scalar.activation(z[:w], pt[:w, q, 0:256], AF.Copy,
                                     scale=dgi_t[:w, t:t + 1])
                u = elu_u(z[:w], w, 256, f"ne{l}", t)
                h = sbuf.tile([P, 256], F32, tag="h", bufs=2, name=f"h{l}_{t}")
                nc.vector.tensor_add(h[:w], u[:w], xinit_sb[:w, t, :])
                for hi in range(2):
                    pT = psum.tile([P, P], F32, tag="pT", bufs=2,
                                   name=f"hpT{l}_{t}_{hi}")
                    nc.tensor.transpose(out=pT[:, 0:w],
                                        in_=h[:w, hi * P:(hi + 1) * P],
                                        identity=ident[:w, :w])
                    nc.vector.tensor_copy(
                        out=hT[hi][:, t * P:t * P + w], in_=pT[:, 0:w])

            scatter_pass(
                [dict(in_ap=ytab[:], es=ESE, idx_t=geai_t,
                      ec_t=ecea_t, cpt=meta["cpt_e2v_a"],
                      regs=meta["regs_ea"], tag=f"ea{l}", gb=GB_E, ring="gea")],
                256, NT_V, e2v_tile)

        # layer 0
        def l0_of(t, w):
            return [xt_t[:, t * P:t * P + w]]

        dphgnn(0, l0_of)

        # layer 1
        def l1_of(t, w):
            return [hT[0][:, t * P:t * P + w], hT[1][:, t * P:t * P + w]]

        dphgnn(1, l1_of)

        # hyperconv
        table2 = dram.tile([NT_V * P, ESH], BF16, name="T2")
        dense_and_table(2, l1_of, table2, False)

        part3 = dram.tile([ME, 128], BF16, name="part3")
        rse3 = dram.tile([MS_OWN, 128], BF16, name="rse3")
        yin3 = dram.tile([MS_OWN, 128], BF16, name="yin3")
        ytab3 = dram.tile([ME, 128], BF16, name="ytab3")
        pstg3 = [None]
        rs3_done = [0]

        def v2e3_tile(t, pt, q, last_in_w, n_in_w):
            if pstg3[0] is None:
                pstg3[0] = sbuf.tile([P, PSW, 128], BF16, tag="pstg",
                                     bufs=4, name=f"pstg3_{t}")
            nc.vector.tensor_copy(out=pstg3[0][:, q, :], in_=pt[:, q, 0:128])
            if last_in_w:
                rows0 = (t - n_in_w + 1) * P
                nc.sync.dma_start(
                    out=part3[rows0:rows0 + n_in_w * P, :].rearrange(
                        "(j p) c -> p j c", p=P),
                    in_=pstg3[0][:, 0:n_in_w, :])
                pstg3[0] = None
            if last_in_w:
                while (rs3_done[0] < RSK
                       and t >= (rs3_done[0] + 1) * CHT - 1 + RS_LAG):
                    kc = rs3_done[0]
                    cc_emit("ReduceScatter", mybir.AluOpType.add,
                            [part3[kc * CHROWS:(kc + 1) * CHROWS, :]],
                            [rse3[kc * OWNR:(kc + 1) * OWNR, :]])
                    rs3_done[0] += 1

        scatter_pass([dict(in_ap=table2[:], es=ESH, idx_t=gvi_t,
                           ec_t=ecv_t, cpt=meta["cpt_v2e"],
                           regs=meta["regs_v"], tag="v2", gb=GB_V, ring="gv")],
                     128, NT_E, v2e3_tile)
        while rs3_done[0] < RSK:
            kc = rs3_done[0]
            cc_emit("ReduceScatter", mybir.AluOpType.add,
                    [part3[kc * CHROWS:(kc + 1) * CHROWS, :]],
                    [rse3[kc * OWNR:(kc + 1) * OWNR, :]])
            rs3_done[0] += 1

        ag3_done = [0]
        for g in range(NT_MS):
            r0 = g * P
            rt = sbuf.tile([P, 128], BF16, tag="rt3", bufs=2,
                           name=f"rt3_{g}")
            nc.scalar.dma_start(rt[:], rse3[r0:r0 + P, :])
            yt = sbuf.tile([P, 128], BF16, tag="yt3", bufs=2,
                           name=f"yt3_{g}")
            nc.vector.tensor_scalar_mul(out=yt[:], in0=rt[:],
                                        scalar1=cf_t[:, g:g + 1])
            nc.sync.dma_start(out=yin3[r0:r0 + P, :], in_=yt[:])
            while (ag3_done[0] < RSK
                   and (g + 1) * P >= (ag3_done[0] + 1) * OWNR):
                k = ag3_done[0]
                cc_emit("AllGather", mybir.AluOpType.bypass,
                        [yin3[k * OWNR:(k + 1) * OWNR, :]],
                        [ytab3[k * CHROWS:(k + 1) * CHROWS, :]])
                ag3_done[0] += 1

        def e2v3_tile(t, pt, q, last_in_w, n_in_w):
            w = min(P, NS - t * P)
            ot = sbuf.tile([P, 128], F32, tag="fo", bufs=2, name=f"fo_{t}")
            nc.vector.tensor_scalar_mul(out=ot[:w], in0=pt[:w, q, 0:128],
                                        scalar1=dvi_t[:w, t:t + 1])
            nc.sync.dma_start(out=yout[t * P:t * P + w, :], in_=ot[:w])

        scatter_pass(
            [dict(in_ap=ytab3[:], es=ESH, idx_t=geai_t,
                  ec_t=ecea_t, cpt=meta["cpt_e2v_a"],
                  regs=meta["regs_ea"], tag="e3a", gb=GB_E, ring="gea")],
            128, NT_V, e2v3_tile)
        ctx.close()

    nc.compile()
    return nc


_CACHED = {}


def kernel(**inputs):
    in_maps, meta = _prep(inputs)
    key = (meta["LvP"], meta["LeAP"], meta["LeBP"], tuple(meta["cpt_v2e"]),
           tuple(meta["cpt_e2v_a"]), tuple(meta["cpt_e2v_b"]))
    if key not in _CACHED:
        _CACHED[key] = build_program(meta)
    nc = _CACHED[key]
    res = run_bass_kernel_spmd(nc, in_maps, list(range(NCORES)))
    out = np.concatenate([res.results[c]["yout"] for c in range(NCORES)],
                         axis=0)
    return np.ascontiguousarray(out.astype(np.float32))
